# revision 14
# baseline (speedup 1.0000x reference)
"""Trainium2 Bass kernel for nn_CategoryMultiplier.

out[b, s, :] = inputs[b, s, :] * (emb_table[categories[b, s]] if
               categories[b, s] != 0 else 1.0)

Sharding: pure data parallel over batch. 8 cores x 16 batches each.
Per core: x flat [8192, 512] f32, cats (int16 permuted+wrapped gather
indices + int32 partition-major mask source), table [1000, 512] f32.

Device layout: positions are partition-major (partition p holds positions
p*64 .. p*64+63) so input/output DMAs use 16KB-contiguous descriptors per
partition. Embedding rows are fetched straight from the UNMODIFIED input
table with InstDMAGatherAnt (1 call per chunk, ~10ns/row of Q7 time); its
fixed dst layout dst[i%128, i//128] is reconciled with the partition-major
layout by permuting the index array on the host (pure layout prep).

Padding (category 0 -> multiplier 1.0) is fixed AFTER the gather on the
otherwise-idle Scalar (ACT) engine: within one position-column, the
valid/pad flags are per-partition scalars, so one fused
activation(Copy, scale=valid, bias=is_pad) per column rewrites
g = g*valid + is_pad (garbage row-0 data becomes exactly 1.0). This avoids
any table copy in DRAM - the gathers depend only on the tiny index load,
so the pipeline starts immediately.
"""

import numpy as np

import concourse.bass as bass
import concourse.bacc as bacc
import concourse.mybir as mybir
import concourse.tile as tile
from concourse.bass_utils import run_bass_kernel_spmd

# Problem shape (hardcoded per harness contract).
B, S, D = 128, 512, 512
VOCAB = 1000
N_CORES = 8
B_LOC = B // N_CORES            # 16 batches per core
N = B_LOC * S                   # 8192 positions per core
P = 128                         # SBUF partitions
C = N // P                      # 64 positions per partition
T_CH = 8                        # max positions-per-partition per chunk

F32 = mybir.dt.float32
I32 = mybir.dt.int32
I16 = mybir.dt.int16

# Taper: small chunks at head (prime the pipeline) and tail (short drain).
CHUNKS = [4, 4] + [8] * 6 + [4, 4]
assert sum(CHUNKS) == C


def _build_nc():
    nc = bacc.Bacc("TRN2", target_bir_lowering=False, debug=False)

    x = nc.dram_tensor("x", [N, D], F32, kind="ExternalInput")
    cats16 = nc.dram_tensor("cats16", [P, N // 16], I16, kind="ExternalInput")
    cats32 = nc.dram_tensor("cats32", [P, C], I32, kind="ExternalInput")
    table = nc.dram_tensor("table", [VOCAB, D], F32, kind="ExternalInput")
    y = nc.dram_tensor("y", [N, D], F32, kind="ExternalOutput")

    xr = x[:].rearrange("(p c) d -> p (c d)", p=P)     # [128, C*D]
    yr = y[:].rearrange("(p c) d -> p (c d)", p=P)

    COPY = mybir.ActivationFunctionType.Copy

    with tile.TileContext(nc) as tc:
        with (
            tc.tile_pool(name="const", bufs=1) as const_pool,
            tc.tile_pool(name="io", bufs=4) as io_pool,
            tc.tile_pool(name="gat", bufs=4) as gat_pool,
        ):
            # Gather indices (the only gather prerequisite - tiny).
            cats_t = const_pool.tile([P, N // 16], I16)
            nc.scalar.dma_start(out=cats_t[:], in_=cats16[:])

            # Per-position pad flags: eqf = (cat == 0), validf = 1 - eqf.
            cats32_t = const_pool.tile([P, C], I32)
            nc.scalar.dma_start(out=cats32_t[:], in_=cats32[:])
            catsf = const_pool.tile([P, C], F32)
            nc.vector.tensor_copy(out=catsf[:], in_=cats32_t[:])
            eqf = const_pool.tile([P, C], F32)
            nc.vector.tensor_scalar(
                out=eqf[:], in0=catsf[:], scalar1=0.0, scalar2=None,
                op0=mybir.AluOpType.is_equal,
            )
            validf = const_pool.tile([P, C], F32)
            nc.vector.tensor_scalar(
                out=validf[:], in0=eqf[:], scalar1=-1.0, scalar2=1.0,
                op0=mybir.AluOpType.mult, op1=mybir.AluOpType.add,
            )

            pos = 0
            for tch in CHUNKS:
                lo, hi = pos * D, (pos + tch) * D
                n_idx = tch * P
                g_t = gat_pool.tile([P, T_CH * D], F32, tag="g")
                nc.gpsimd.dma_gather(
                    out_ap=g_t[:, :tch * D].rearrange("p (t d) -> p t d", t=tch),
                    in_ap=table[:],
                    idxs_ap=cats_t[:, pos * 8:(pos + tch) * 8],
                    num_idxs=n_idx,
                    num_idxs_reg=n_idx,
                    elem_size=D,
                )

                x_t = io_pool.tile([P, T_CH * D], F32, tag="x")
                nc.sync.dma_start(out=x_t[:, :tch * D], in_=xr[:, lo:hi])

                # Pad fix: per column, g = g*valid + is_pad (one fused DVE
                # tensor_scalar with per-partition AP scalars).
                for t in range(tch):
                    col = g_t[:, t * D:(t + 1) * D]
                    nc.vector.tensor_scalar(
                        out=col, in0=col,
                        scalar1=validf[:, pos + t:pos + t + 1],
                        scalar2=eqf[:, pos + t:pos + t + 1],
                        op0=mybir.AluOpType.mult, op1=mybir.AluOpType.add,
                    )

                nc.vector.tensor_mul(out=g_t[:, :tch * D], in0=g_t[:, :tch * D],
                                     in1=x_t[:, :tch * D])
                nc.sync.dma_start(out=yr[:, lo:hi], in_=g_t[:, :tch * D])
                pos += tch

    nc.compile()
    return nc


_NC = None


def _get_nc():
    global _NC
    if _NC is None:
        _NC = _build_nc()
    return _NC


def _permute_cats(c):
    """Build the dma_gather index stream for the partition-major layout.

    Stream index s = col*128 + p (col = global position-per-partition)
    must hold cats[p*C + col]. Wrap (index s at [s%16, s//16]) and
    replicate across the 8 16-partition groups.
    """
    a = np.ascontiguousarray(c.reshape(P, C).T).reshape(N)   # [col, p] flat
    return np.ascontiguousarray(np.tile(a.reshape(N // 16, 16).T, (8, 1)))


def _shard_inputs(inputs, categories, emb_table):
    tab = np.ascontiguousarray(emb_table, dtype=np.float32)
    in_maps = []
    for i in range(N_CORES):
        xs = np.ascontiguousarray(
            inputs[i * B_LOC:(i + 1) * B_LOC], dtype=np.float32
        ).reshape(N, D)
        c = categories[i * B_LOC:(i + 1) * B_LOC].reshape(N)
        in_maps.append({
            "x": xs,
            "cats16": _permute_cats(c.astype(np.int16)),
            "cats32": np.ascontiguousarray(c.reshape(P, C).astype(np.int32)),
            "table": tab,
        })
    return in_maps


def kernel(inputs, categories, mask_positions=None, emb_table=None, **_):
    """Full (unsharded) inputs in, full output out. mask_positions unused."""
    nc = _get_nc()
    in_maps = _shard_inputs(inputs, categories, emb_table)
    res = run_bass_kernel_spmd(nc, in_maps, list(range(N_CORES)))
    out = np.empty((B, S, D), dtype=np.float32)
    for i in range(N_CORES):
        out[i * B_LOC:(i + 1) * B_LOC] = res.results[i]["y"].reshape(B_LOC, S, D)
    return out


# revision 15
# speedup vs baseline: 1.1918x; 1.1918x over previous
"""Trainium2 Bass kernel for nn_CategoryMultiplier.

out[b, s, :] = inputs[b, s, :] * (emb_table[categories[b, s]] if
               categories[b, s] != 0 else 1.0)

Sharding: pure data parallel over batch. 8 cores x 16 batches each.
Per core: x flat [8192, 512] f32, cats (int16 permuted + wrapped for
dma_gather), table [1000, 512] f32.

Device layout: positions are partition-major (partition p holds positions
p*64 .. p*64+63) so the input/output DMAs use 16KB-contiguous descriptors
per partition (HWDGE emission is ~6ns/descriptor on the issuing engine).
Embedding rows are fetched with one InstDMAGatherAnt per chunk
(~10ns/row of Q7 time); its fixed dst layout dst[i%128, i//128] is
reconciled with the partition-major layout by permuting the index array
on the host (pure layout prep).

Padding (category 0 -> multiplier 1.0): internal table copy (table2) whose
row 0 is all-ones; row 0 is only gathered by padding positions. The copy
is the gathers' only big prerequisite, so the x-loads are explicitly
ordered after it (ordering-only deps) to give it full ring bandwidth.
"""

import numpy as np

import concourse.bass as bass
import concourse.bacc as bacc
import concourse.mybir as mybir
import concourse.tile as tile
from concourse.tile import add_dep_helper
from concourse.bass_utils import run_bass_kernel_spmd

# Problem shape (hardcoded per harness contract).
B, S, D = 128, 512, 512
VOCAB = 1000
N_CORES = 8
B_LOC = B // N_CORES            # 16 batches per core
N = B_LOC * S                   # 8192 positions per core
P = 128                         # SBUF partitions
C = N // P                      # 64 positions per partition
T_CH = 8                        # max positions-per-partition per chunk

F32 = mybir.dt.float32
I16 = mybir.dt.int16

# Taper: small chunks at head (prime the pipeline) and tail (short drain).
CHUNKS = [4, 4] + [8] * 6 + [4, 4]
assert sum(CHUNKS) == C


def _build_nc():
    nc = bacc.Bacc("TRN2", target_bir_lowering=False, debug=False)

    x = nc.dram_tensor("x", [N, D], F32, kind="ExternalInput")
    cats16 = nc.dram_tensor("cats16", [P, N // 16], I16, kind="ExternalInput")
    table = nc.dram_tensor("table", [VOCAB, D], F32, kind="ExternalInput")
    y = nc.dram_tensor("y", [N, D], F32, kind="ExternalOutput")
    table2 = nc.dram_tensor("table2", [VOCAB, D], F32)

    xr = x[:].rearrange("(p c) d -> p (c d)", p=P)     # [128, C*D]
    yr = y[:].rearrange("(p c) d -> p (c d)", p=P)
    # Flat views so the table copy lowers to few large descriptors.
    tf = table[:].rearrange("v d -> (v d)")
    t2f = table2[:].rearrange("v d -> (v d)")
    HALF = (VOCAB // 2) * D

    with tile.TileContext(nc) as tc:
        with (
            tc.tile_pool(name="const", bufs=1) as const_pool,
            tc.tile_pool(name="io", bufs=4) as io_pool,
            tc.tile_pool(name="gat", bufs=4) as gat_pool,
        ):
            # Tiny prerequisites on the ACT ring.
            cats_t = const_pool.tile([P, N // 16], I16)
            nc.scalar.dma_start(out=cats_t[:], in_=cats16[:])
            ones = const_pool.tile([1, D], F32)
            nc.gpsimd.memset(ones[:], 1.0)
            nc.scalar.dma_start(out=table2[0:1, :], in_=ones[:])

            # table2 rows 1.. on the SP ring, ahead of all x-loads.
            t2a = nc.sync.dma_start(out=t2f[D:HALF], in_=tf[D:HALF])
            t2b = nc.sync.dma_start(out=t2f[HALF:], in_=tf[HALF:])

            pos = 0
            for ci, tch in enumerate(CHUNKS):
                lo, hi = pos * D, (pos + tch) * D
                n_idx = tch * P
                g_t = gat_pool.tile([P, T_CH * D], F32, tag="g")
                nc.gpsimd.dma_gather(
                    out_ap=g_t[:, :tch * D].rearrange("p (t d) -> p t d", t=tch),
                    in_ap=table2[:],
                    idxs_ap=cats_t[:, pos * 8:(pos + tch) * 8],
                    num_idxs=n_idx,
                    num_idxs_reg=n_idx,
                    elem_size=D,
                )

                x_t = io_pool.tile([P, T_CH * D], F32, tag="x")
                xi = nc.sync.dma_start(out=x_t[:, :tch * D], in_=xr[:, lo:hi])
                if ci < 4:
                    # Ordering-only: keep the table copy at the SP ring head.
                    add_dep_helper(xi.ins, t2a.ins, sync=False,
                                   reason="x-load after table2 copy")
                    add_dep_helper(xi.ins, t2b.ins, sync=False,
                                   reason="x-load after table2 copy")

                nc.vector.tensor_mul(out=g_t[:, :tch * D], in0=g_t[:, :tch * D],
                                     in1=x_t[:, :tch * D])
                nc.scalar.dma_start(out=yr[:, lo:hi], in_=g_t[:, :tch * D])
                pos += tch

    nc.compile()
    return nc


_NC = None


def _get_nc():
    global _NC
    if _NC is None:
        _NC = _build_nc()
    return _NC


def _permute_cats(c):
    """Build the dma_gather index stream for the partition-major layout.

    Stream index s = col*128 + p (col = global position-per-partition)
    must hold cats[p*C + col]. Wrap (index s at [s%16, s//16]) and
    replicate across the 8 16-partition groups.
    """
    a = np.ascontiguousarray(c.reshape(P, C).T).reshape(N)   # [col, p] flat
    return np.ascontiguousarray(np.tile(a.reshape(N // 16, 16).T, (8, 1)))


def _shard_inputs(inputs, categories, emb_table):
    tab = np.ascontiguousarray(emb_table, dtype=np.float32)
    in_maps = []
    for i in range(N_CORES):
        xs = np.ascontiguousarray(
            inputs[i * B_LOC:(i + 1) * B_LOC], dtype=np.float32
        ).reshape(N, D)
        c = categories[i * B_LOC:(i + 1) * B_LOC].reshape(N).astype(np.int16)
        in_maps.append({"x": xs, "cats16": _permute_cats(c), "table": tab})
    return in_maps


def kernel(inputs, categories, mask_positions=None, emb_table=None, **_):
    """Full (unsharded) inputs in, full output out. mask_positions unused."""
    nc = _get_nc()
    in_maps = _shard_inputs(inputs, categories, emb_table)
    res = run_bass_kernel_spmd(nc, in_maps, list(range(N_CORES)))
    out = np.empty((B, S, D), dtype=np.float32)
    for i in range(N_CORES):
        out[i * B_LOC:(i + 1) * B_LOC] = res.results[i]["y"].reshape(B_LOC, S, D)
    return out
